# revision 10
# baseline (speedup 1.0000x reference)
"""Trainium2 Bass kernel for the Beta spike-slab linear layer (no bias).

Computation (training mode):
    p         = sigmoid(p_logit)                       [D]
    var_beta  = softplus(rho_beta)                     [D,1]
    var_sigma = softplus(rho_logsigma)                 [D]
    beta      = mu_beta + var_beta * eps_beta          [D,1]
    sigma_vec = exp(mu_logsigma + var_sigma*eps_sigma) [D]
    keep_prob = sigmoid(hard_concrete(p, unif)/TEMP)   [D]
    output    = (x * (keep_prob*sigma_vec)) @ beta     [N,1]
    kl        = scalar KL terms over the [D] params    [1]

Since O == 1, the heavy part collapses to a mat-vec: out = x @ c with
c[d] = keep_prob[d]*sigma_vec[d]*beta[d,0].  x is [262144, 1024] f32
(1 GiB) -> memory-bound.  Strategy: data-parallel over rows across the
8 NeuronCores (128 MiB each), replicate the tiny coefficient vector.
The [D]-sized coefficient/KL math is O(1024) scalar work, computed on
host in float64 (more accurate than the f32 reference).

Per-core device kernel: stream x tiles [128 part x R rows x 1024] into
SBUF; one fused DVE scalar_tensor_tensor per row computes the dot
product with the broadcast coefficient vector (accum_out reduces along
the free axis); results accumulate in an SBUF [128, J] tile streamed
back in chunks.  Row n_local = p*J + j lives in partition p, column j,
so the output tile is exactly out_shard.reshape(128, J).  Small
prologue/epilogue DMA tiles shorten pipeline fill and drain.

Measured on trn2 (8 cores, NTFF profile): ~353 us on a quiet machine,
~134 MB/core at ~408 GB/s active DMA — at the HBM roofline; DVE busy
~88% (317 us), all other engines idle.
"""

import math
from contextlib import ExitStack

import numpy as np

import concourse.bass as bass
import concourse.tile as tile
from concourse import bacc, mybir
from concourse.bass_utils import run_bass_kernel_spmd

# Problem constants (hardcoded per the harness contract).
N_TOTAL = 262144
D = 1024
O = 1
N_CORES = 8
P = 128  # SBUF partitions

SIGMA = 0.6
ALPHA_B, BETA_B = 1.0, 1.0
ALPHA_G, BETA_G = 1.0, 0.5
EPS = 1e-7
EPS_HC = 1e-4
TEMP = 0.3

# Set by test harness to capture profile info.
TRACE = False
LAST_RESULTS = None

_BUILD_CACHE = {}


def _tile_specs(j_cols: int, rows_per_tile: int):
    """(start_j, n_rows) DMA tiles: small prologue tiles so compute starts
    early, small epilogue tiles so the pipeline drains fast."""
    if j_cols <= 4 * rows_per_tile or rows_per_tile < 4:
        return [(j, rows_per_tile)
                for j in range(0, j_cols, rows_per_tile)]
    pro = [1, 1, 2]
    epi = [2, 1, 1]
    specs = []
    j = 0
    for n in pro:
        specs.append((j, n)); j += n
    end_j = j_cols - sum(epi)
    while j < end_j:
        n = min(rows_per_tile, end_j - j)
        specs.append((j, n)); j += n
    for n in epi:
        specs.append((j, n)); j += n
    assert j == j_cols
    return specs


def build_matvec(rows_per_core: int, d: int = D, rows_per_tile: int = 4,
                 bufs: int = 8, out_chunks: int = 4):
    """Bass program: out[p, j] = sum_d x[p, j, d] * c[0, d]."""
    assert rows_per_core % P == 0
    j_cols = rows_per_core // P
    f32 = mybir.dt.float32

    nc = bacc.Bacc("TRN2", target_bir_lowering=False)
    x_h = nc.dram_tensor("x", [P, j_cols, d], f32, kind="ExternalInput")
    c_h = nc.dram_tensor("c", [1, d], f32, kind="ExternalInput")
    o_h = nc.dram_tensor("out", [P, j_cols], f32, kind="ExternalOutput")

    with tile.TileContext(nc) as tc, ExitStack() as ctx:
        singles = ctx.enter_context(tc.tile_pool(name="singles", bufs=1))
        xpool = ctx.enter_context(tc.tile_pool(name="xp", bufs=bufs))

        c_sb = singles.tile([P, d], f32)
        nc.sync.dma_start(out=c_sb[:], in_=c_h[:].to_broadcast([P, d]))
        out_sb = singles.tile([P, j_cols], f32)
        scratch = singles.tile([P, d], f32)

        chunk = max(1, j_cols // out_chunks)
        next_out = chunk
        # alternate the two HWDGE rings (sync + scalar) so each SDMA
        # engine round-robins two queues' packets
        rings = [nc.sync, nc.scalar]
        for ti, (j0, n_rows) in enumerate(_tile_specs(j_cols, rows_per_tile)):
            xt = xpool.tile([P, rows_per_tile, d], f32, tag="xt")
            rings[ti % 2].dma_start(
                out=xt[:, :n_rows, :],
                in_=x_h[:, j0:j0 + n_rows, :],
            )
            for r in range(n_rows):
                j = j0 + r
                nc.vector.scalar_tensor_tensor(
                    out=scratch[:],
                    in0=xt[:, r, :],
                    scalar=1.0,
                    in1=c_sb[:],
                    op0=mybir.AluOpType.mult,
                    op1=mybir.AluOpType.mult,
                    accum_out=out_sb[:, j:j + 1],
                )
            # stream finished output columns out as we go
            while next_out <= j0 + n_rows and next_out <= j_cols:
                a = next_out - chunk
                b = j_cols if j_cols - next_out < chunk else next_out
                nc.sync.dma_start(out=o_h[:, a:b], in_=out_sb[:, a:b])
                next_out = b + chunk if b < j_cols else j_cols + chunk
    nc.compile()
    return nc


def _get_program(rows_per_core: int):
    key = rows_per_core
    if key not in _BUILD_CACHE:
        _BUILD_CACHE[key] = build_matvec(rows_per_core)
    return _BUILD_CACHE[key]


def _sigmoid(x):
    return 1.0 / (1.0 + np.exp(-x))


def _softplus(x):
    return np.log1p(np.exp(-np.abs(x))) + np.maximum(x, 0.0)


def _host_coeff_and_kl(training, eps_beta, eps_sigma, unif_noise, mu_beta,
                       rho_beta, p_logit, mu_logsigma, rho_logsigma,
                       pi_posterior):
    """Coefficient vector c [D] and the scalar KL [1], in float64."""
    p_logit = np.asarray(p_logit, np.float64)
    mu_beta = np.asarray(mu_beta, np.float64)
    rho_beta = np.asarray(rho_beta, np.float64)
    mu_logsigma = np.asarray(mu_logsigma, np.float64)
    rho_logsigma = np.asarray(rho_logsigma, np.float64)
    eps_beta = np.asarray(eps_beta, np.float64)
    eps_sigma = np.asarray(eps_sigma, np.float64)
    unif_noise = np.asarray(unif_noise, np.float64)
    pi = np.asarray(pi_posterior, np.float64)

    p = _sigmoid(p_logit)                      # [D]
    var_beta = _softplus(rho_beta)             # [D,1]
    var_sigma = _softplus(rho_logsigma)        # [D]

    if training:
        beta = mu_beta + var_beta * eps_beta                      # [D,1]
        sigma_vec = np.exp(mu_logsigma + var_sigma * eps_sigma)   # [D]
        s = (np.log(p + EPS_HC) - np.log(1.0 - p + EPS_HC)
             + np.log(unif_noise + EPS_HC) - np.log(1.0 - unif_noise + EPS_HC))
        keep_prob = _sigmoid(s / TEMP)                            # [D]
        c = keep_prob * sigma_vec * beta[:, 0]                    # [D]
    else:
        c = p * np.exp(mu_logsigma) * mu_beta[:, 0]

    kl_beta = np.sum((var_beta ** 2 + mu_beta ** 2) / (2.0 * SIGMA ** 2)
                     - np.log(var_beta + EPS) + np.log(SIGMA) - 0.5)
    kl_entropy = np.sum(p * np.log(p + EPS)
                        + (1.0 - p) * np.log(1.0 - p + EPS))
    exp_prior = -np.sum(p * np.log(pi + EPS)
                        + (1.0 - p) * np.log(1.0 - pi + EPS))
    exp_beta = ((ALPHA_B - 1.0) * np.log(pi + EPS)
                + (BETA_B - 1.0) * np.log(1.0 - pi + EPS)
                - math.lgamma(ALPHA_B) - math.lgamma(BETA_B)
                + math.lgamma(ALPHA_B + BETA_B))
    kl_spike = kl_entropy + exp_prior - exp_beta  # [1]
    entropy_sigma = np.sum(mu_logsigma + np.log(var_sigma)
                           + 0.5 * np.log(2.0 * np.pi) + 0.5)
    prior_sigma = np.sum(ALPHA_G * np.log(BETA_G)
                         - (ALPHA_G + 1.0) * mu_logsigma
                         - BETA_G * np.exp(-mu_logsigma + 0.5 * var_sigma ** 2)
                         - math.lgamma(ALPHA_G))
    kl_scale = -entropy_sigma - prior_sigma
    kl = kl_beta + kl_spike + kl_scale
    return c, np.asarray(kl, np.float64).reshape(1)


def kernel(x, training, eps_beta, eps_sigma, unif_noise, mu_beta, rho_beta,
           p_logit, mu_logsigma, rho_logsigma, pi_posterior):
    global LAST_RESULTS
    x = np.ascontiguousarray(np.asarray(x, np.float32))
    n = x.shape[0]
    training_flag = bool(np.asarray(training).reshape(-1)[0] != 0) \
        if np.asarray(training).size else bool(training)

    c64, kl64 = _host_coeff_and_kl(
        training_flag, eps_beta, eps_sigma, unif_noise, mu_beta, rho_beta,
        p_logit, mu_logsigma, rho_logsigma, pi_posterior)
    c = np.ascontiguousarray(c64.astype(np.float32).reshape(1, D))
    kl = kl64.astype(np.float32)

    assert n % N_CORES == 0
    rows_per_core = n // N_CORES
    j_cols = rows_per_core // P
    nc = _get_program(rows_per_core)

    in_maps = []
    for i in range(N_CORES):
        shard = x[i * rows_per_core:(i + 1) * rows_per_core]
        in_maps.append({"x": shard.reshape(P, j_cols, D), "c": c})

    try:
        res = run_bass_kernel_spmd(nc, in_maps, list(range(N_CORES)),
                                   trace=TRACE)
    except Exception:
        # transient terminal/device failures have been observed; retry once
        res = run_bass_kernel_spmd(nc, in_maps, list(range(N_CORES)),
                                   trace=TRACE)
    LAST_RESULTS = res
    out = np.concatenate(
        [res.results[i]["out"].reshape(-1) for i in range(N_CORES)]
    ).astype(np.float32).reshape(n, O)
    return out, kl


# revision 13
# speedup vs baseline: 1.2815x; 1.2815x over previous
"""Trainium2 Bass kernel for the Beta spike-slab linear layer (no bias).

Computation (training mode):
    p         = sigmoid(p_logit)                       [D]
    var_beta  = softplus(rho_beta)                     [D,1]
    var_sigma = softplus(rho_logsigma)                 [D]
    beta      = mu_beta + var_beta * eps_beta          [D,1]
    sigma_vec = exp(mu_logsigma + var_sigma*eps_sigma) [D]
    keep_prob = sigmoid(hard_concrete(p, unif)/TEMP)   [D]
    output    = (x * (keep_prob*sigma_vec)) @ beta     [N,1]
    kl        = scalar KL terms over the [D] params    [1]

Since O == 1, the heavy part collapses to a mat-vec: out = x @ c with
c[d] = keep_prob[d]*sigma_vec[d]*beta[d,0].  x is [262144, 1024] f32
(1 GiB) -> memory-bound.  Strategy: data-parallel over rows across the
8 NeuronCores (128 MiB each), replicate the tiny coefficient vector.
The [D]-sized coefficient/KL math is O(1024) scalar work, computed on
host in float64 (more accurate than the f32 reference).

Per-core device kernel: stream x tiles [128 part x R rows x 1024] into
SBUF; one fused DVE scalar_tensor_tensor per row computes the dot
product with the broadcast coefficient vector (accum_out reduces along
the free axis); results accumulate in an SBUF [128, J] tile streamed
back in chunks.  Row n_local = p*J + j lives in partition p, column j,
so the output tile is exactly out_shard.reshape(128, J).  Small
prologue/epilogue DMA tiles shorten pipeline fill and drain.

Measured on trn2 (8 cores, NTFF profile): ~353 us on a quiet machine,
~134 MB/core at ~408 GB/s active DMA — at the HBM roofline; DVE busy
~88% (317 us), all other engines idle.
"""

import math
from contextlib import ExitStack

import numpy as np

import concourse.bass as bass
import concourse.tile as tile
from concourse import bacc, mybir
from concourse.bass_utils import run_bass_kernel_spmd

# Problem constants (hardcoded per the harness contract).
N_TOTAL = 262144
D = 1024
O = 1
N_CORES = 8
P = 128  # SBUF partitions

SIGMA = 0.6
ALPHA_B, BETA_B = 1.0, 1.0
ALPHA_G, BETA_G = 1.0, 0.5
EPS = 1e-7
EPS_HC = 1e-4
TEMP = 0.3

# Set by test harness to capture profile info.
TRACE = False
LAST_RESULTS = None

_BUILD_CACHE = {}


def _tile_specs(j_cols: int, rows_per_tile: int):
    """(start_j, n_rows) DMA tiles: small prologue tiles so compute starts
    early, small epilogue tiles so the pipeline drains fast."""
    if j_cols <= 4 * rows_per_tile or rows_per_tile < 4:
        return [(j, rows_per_tile)
                for j in range(0, j_cols, rows_per_tile)]
    pro = [1, 1, 2]
    epi = [2, 1, 1]
    specs = []
    j = 0
    for n in pro:
        specs.append((j, n)); j += n
    end_j = j_cols - sum(epi)
    while j < end_j:
        n = min(rows_per_tile, end_j - j)
        specs.append((j, n)); j += n
    for n in epi:
        specs.append((j, n)); j += n
    assert j == j_cols
    return specs


def build_matvec(rows_per_core: int, d: int = D, rows_per_tile: int = 4,
                 bufs: int = 8, out_chunks: int = 4):
    """Bass program: out[p, j] = sum_d x[p, j, d] * c[0, d]."""
    assert rows_per_core % P == 0
    j_cols = rows_per_core // P
    f32 = mybir.dt.float32

    nc = bacc.Bacc("TRN2", target_bir_lowering=False)
    x_h = nc.dram_tensor("x", [P, j_cols, d], f32, kind="ExternalInput")
    c_h = nc.dram_tensor("c", [1, d], f32, kind="ExternalInput")
    o_h = nc.dram_tensor("out", [P, j_cols], f32, kind="ExternalOutput")

    with tile.TileContext(nc) as tc, ExitStack() as ctx:
        singles = ctx.enter_context(tc.tile_pool(name="singles", bufs=1))
        xpool = ctx.enter_context(tc.tile_pool(name="xp", bufs=bufs))

        c_sb = singles.tile([P, d], f32)
        nc.sync.dma_start(out=c_sb[:], in_=c_h[:].to_broadcast([P, d]))
        out_sb = singles.tile([P, j_cols], f32)
        scratch = singles.tile([P, d], f32)

        chunk = max(1, j_cols // out_chunks)
        next_out = chunk
        for (j0, n_rows) in _tile_specs(j_cols, rows_per_tile):
            xt = xpool.tile([P, rows_per_tile, d], f32, tag="xt")
            nc.sync.dma_start(
                out=xt[:, :n_rows, :],
                in_=x_h[:, j0:j0 + n_rows, :],
            )
            for r in range(n_rows):
                j = j0 + r
                nc.vector.scalar_tensor_tensor(
                    out=scratch[:],
                    in0=xt[:, r, :],
                    scalar=1.0,
                    in1=c_sb[:],
                    op0=mybir.AluOpType.mult,
                    op1=mybir.AluOpType.mult,
                    accum_out=out_sb[:, j:j + 1],
                )
            # stream finished output columns out as we go
            while next_out <= j0 + n_rows and next_out <= j_cols:
                a = next_out - chunk
                b = j_cols if j_cols - next_out < chunk else next_out
                nc.sync.dma_start(out=o_h[:, a:b], in_=out_sb[:, a:b])
                next_out = b + chunk if b < j_cols else j_cols + chunk
    nc.compile()
    return nc


def _get_program(rows_per_core: int):
    key = rows_per_core
    if key not in _BUILD_CACHE:
        _BUILD_CACHE[key] = build_matvec(rows_per_core)
    return _BUILD_CACHE[key]


def _sigmoid(x):
    return 1.0 / (1.0 + np.exp(-x))


def _softplus(x):
    return np.log1p(np.exp(-np.abs(x))) + np.maximum(x, 0.0)


def _host_coeff_and_kl(training, eps_beta, eps_sigma, unif_noise, mu_beta,
                       rho_beta, p_logit, mu_logsigma, rho_logsigma,
                       pi_posterior):
    """Coefficient vector c [D] and the scalar KL [1], in float64."""
    p_logit = np.asarray(p_logit, np.float64)
    mu_beta = np.asarray(mu_beta, np.float64)
    rho_beta = np.asarray(rho_beta, np.float64)
    mu_logsigma = np.asarray(mu_logsigma, np.float64)
    rho_logsigma = np.asarray(rho_logsigma, np.float64)
    eps_beta = np.asarray(eps_beta, np.float64)
    eps_sigma = np.asarray(eps_sigma, np.float64)
    unif_noise = np.asarray(unif_noise, np.float64)
    pi = np.asarray(pi_posterior, np.float64)

    p = _sigmoid(p_logit)                      # [D]
    var_beta = _softplus(rho_beta)             # [D,1]
    var_sigma = _softplus(rho_logsigma)        # [D]

    if training:
        beta = mu_beta + var_beta * eps_beta                      # [D,1]
        sigma_vec = np.exp(mu_logsigma + var_sigma * eps_sigma)   # [D]
        s = (np.log(p + EPS_HC) - np.log(1.0 - p + EPS_HC)
             + np.log(unif_noise + EPS_HC) - np.log(1.0 - unif_noise + EPS_HC))
        keep_prob = _sigmoid(s / TEMP)                            # [D]
        c = keep_prob * sigma_vec * beta[:, 0]                    # [D]
    else:
        c = p * np.exp(mu_logsigma) * mu_beta[:, 0]

    kl_beta = np.sum((var_beta ** 2 + mu_beta ** 2) / (2.0 * SIGMA ** 2)
                     - np.log(var_beta + EPS) + np.log(SIGMA) - 0.5)
    kl_entropy = np.sum(p * np.log(p + EPS)
                        + (1.0 - p) * np.log(1.0 - p + EPS))
    exp_prior = -np.sum(p * np.log(pi + EPS)
                        + (1.0 - p) * np.log(1.0 - pi + EPS))
    exp_beta = ((ALPHA_B - 1.0) * np.log(pi + EPS)
                + (BETA_B - 1.0) * np.log(1.0 - pi + EPS)
                - math.lgamma(ALPHA_B) - math.lgamma(BETA_B)
                + math.lgamma(ALPHA_B + BETA_B))
    kl_spike = kl_entropy + exp_prior - exp_beta  # [1]
    entropy_sigma = np.sum(mu_logsigma + np.log(var_sigma)
                           + 0.5 * np.log(2.0 * np.pi) + 0.5)
    prior_sigma = np.sum(ALPHA_G * np.log(BETA_G)
                         - (ALPHA_G + 1.0) * mu_logsigma
                         - BETA_G * np.exp(-mu_logsigma + 0.5 * var_sigma ** 2)
                         - math.lgamma(ALPHA_G))
    kl_scale = -entropy_sigma - prior_sigma
    kl = kl_beta + kl_spike + kl_scale
    return c, np.asarray(kl, np.float64).reshape(1)


def kernel(x, training, eps_beta, eps_sigma, unif_noise, mu_beta, rho_beta,
           p_logit, mu_logsigma, rho_logsigma, pi_posterior):
    global LAST_RESULTS
    x = np.ascontiguousarray(np.asarray(x, np.float32))
    n = x.shape[0]
    training_flag = bool(np.asarray(training).reshape(-1)[0] != 0) \
        if np.asarray(training).size else bool(training)

    c64, kl64 = _host_coeff_and_kl(
        training_flag, eps_beta, eps_sigma, unif_noise, mu_beta, rho_beta,
        p_logit, mu_logsigma, rho_logsigma, pi_posterior)
    c = np.ascontiguousarray(c64.astype(np.float32).reshape(1, D))
    kl = kl64.astype(np.float32)

    assert n % N_CORES == 0
    rows_per_core = n // N_CORES
    j_cols = rows_per_core // P
    nc = _get_program(rows_per_core)

    in_maps = []
    for i in range(N_CORES):
        shard = x[i * rows_per_core:(i + 1) * rows_per_core]
        in_maps.append({"x": shard.reshape(P, j_cols, D), "c": c})

    try:
        res = run_bass_kernel_spmd(nc, in_maps, list(range(N_CORES)),
                                   trace=TRACE)
    except Exception:
        # transient terminal/device failures have been observed; retry once
        res = run_bass_kernel_spmd(nc, in_maps, list(range(N_CORES)),
                                   trace=TRACE)
    LAST_RESULTS = res
    out = np.concatenate(
        [res.results[i]["out"].reshape(-1) for i in range(N_CORES)]
    ).astype(np.float32).reshape(n, O)
    return out, kl
